# revision 33
# baseline (speedup 1.0000x reference)
"""Trainium2 Bass kernel for nn_AdversarialGenerator (gumbel-sinkhorn permutation).

Contract: kernel(**inputs) takes FULL numpy inputs, returns
(types_permed [B,L] f32, times_permed [B,L] f32, perm [B,L,L] f32).

Strategy (8 NeuronCores, one SPMD launch):
  - Specializes the program on the runtime seq_lens values: only the valid
    s_b x s_b block of each batch is computed; everything else is exactly 0
    (host pads gumbel with -1e30 so exp() kills padding).
  - Phase A: scores + exp, data-parallel over rows across all 8 cores.
    relu rows [H=128, W] on DVE/ACT; the W2-dot runs on the PE via a
    sliding-window masked-weight matrix so score rows accumulate row-major
    in PSUM (f32r for 1 cycle/row).
  - AllGather of the exp(score+gumbel) chunks (ordered with add_dep_helper).
  - Phase B: Sinkhorn in scaling-vector form r=1/(Kc), c=1/(K^T r) -- all
    chunked PE matmuls with [128,k] vector layouts and tiny DVE reciprocals.
    Replicated on every core (cheaper than per-iteration collectives).
  - Phase C: perm = diag(r) K diag(c); type/time einsums on PE; compact
    DMA outputs; host pastes into zeros.
"""
import os
import sys

sys.path.insert(0, "/opt/trn_rl_repo")

from contextlib import ExitStack

import numpy as np
import orjson

import concourse.bass as bass
import concourse.tile as tile
from concourse import mybir
from concourse.bass_utils import run_bass_kernel_spmd
from concourse.tile_rust import add_dep_helper

f32 = mybir.dt.float32
f32r = mybir.dt.float32r
AF = mybir.ActivationFunctionType
OP = mybir.AluOpType

B, L, D, H = 4, 512, 64, 128
TAU = 0.5
ITERS = 10
NCORES = 8
ACT_NUM = 5
ACT_DEN = 11
RELU_BUFS = 8
PHASE_LIMIT = "all"


class _PhaseDone(Exception):
    pass


_build_partial_result = None


def _set_partial(nc):
    global _build_partial_result
    _build_partial_result = (nc, [])

NEG_FILL = -1e30
EPS = 1e-30


# ---------------------------------------------------------------- wait fix
# This container's walrus accepts at most ONE sync wait per instruction.
# Tile attaches several; split the excess onto EventSemaphore carriers
# inserted right before the offender (same engine => same order).
def _legalize_bir_waits(bir: dict, max_waits: int = 1) -> int:
    n = 0
    for func in bir.get("functions", []):
        for bb in func.get("blocks", []):
            out = []
            for ins in bb.get("instructions", []):
                si = ins.get("sync_info")
                waits = (si or {}).get("on_wait") or []
                if len(waits) > max_waits:
                    excess, keep = waits[:-max_waits], waits[-max_waits:]
                    for k, w in enumerate(excess):
                        out.append({
                            "name": f"{ins['name']}_xw{k}",
                            "opcode": "EventSemaphore",
                            "engine": ins["engine"],
                            "ins": [],
                            "outs": [],
                            "sync_info": {"on_wait": [w], "on_update": []},
                            "debug": ins.get("debug"),
                        })
                        n += 1
                    si["on_wait"] = keep
                out.append(ins)
            bb["instructions"] = out
    return n


_patched = False


def _install_wait_fix():
    global _patched
    if _patched:
        return
    _patched = True
    import concourse.bass_utils as bu
    import concourse.bass2jax as b2j

    orig = bu.compile_bir_kernel

    def patched(bir_json, tmpdir, neff_name="file.neff"):
        bir = orjson.loads(bir_json)
        if _legalize_bir_waits(bir):
            bir_json = orjson.dumps(bir)
        return orig(bir_json, tmpdir, neff_name=neff_name)

    bu.compile_bir_kernel = patched
    b2j.compile_bir_kernel = patched


# ---------------------------------------------------------------- layout plan
def _plan(s_list):
    cdiv = lambda a, b: (a + b - 1) // b
    bigs, smalls = [], []
    for b in range(len(s_list)):
        (bigs if s_list[b] > 128 else smalls).append(b)

    plan = {"bigs": [], "smalls": [], "s": list(s_list)}
    for b in bigs:
        s = s_list[b]
        Q = cdiv(s, NCORES)          # rows per core
        R = NCORES * Q               # gathered rows (>= s)
        rt = cdiv(R, 128)            # row tiles
        ct = cdiv(s, 128)            # col tiles
        plan["bigs"].append({"b": b, "s": s, "Q": Q, "R": R, "rt": rt,
                             "ct": ct, "W": ct * 128, "We": 2 * cdiv(s, 2)})
    ns = len(smalls)
    if ns:
        best = None
        import itertools
        for alloc in itertools.product(range(1, NCORES + 1), repeat=ns):
            if sum(alloc) > NCORES:
                continue
            if any(a * cdiv(s_list[b], a) > 128 for a, b in zip(alloc, smalls)):
                continue
            qs = max(cdiv(s_list[b], a) for a, b in zip(alloc, smalls))
            key = (qs, sum(alloc))
            if best is None or key < best[0]:
                best = (key, alloc)
        alloc = list(best[1])
        Qs = max(cdiv(s_list[b], a) for a, b in zip(alloc, smalls))
        # core -> (small index k, chunk index ci); unassigned cores mirror k=0,ci=0
        core_small = [(0, 0)] * NCORES
        cidx = 0
        for k, (a, b) in enumerate(zip(alloc, smalls)):
            for ci in range(a):
                core_small[cidx] = (k, ci)
                cidx += 1
        plan["smalls"] = [{"b": b, "s": s_list[b], "alloc": a}
                          for a, b in zip(alloc, smalls)]
        plan["Qs"] = Qs
        plan["core_small"] = core_small
        plan["ns"] = ns
        plan["WeS"] = 2 * cdiv(max(s_list[b] for b in smalls), 2)
    else:
        plan["WeS"] = 0
        plan["Qs"] = 0
        plan["ns"] = 0
        plan["core_small"] = [(0, 0)] * NCORES
    return plan


# ---------------------------------------------------------------- builder
def _build(plan):
    try:
        return _build_inner(plan)
    except _PhaseDone:
        return _build_partial_result


def _build_inner(plan):
    global _build_partial_result
    nc = bass.Bass(num_devices=NCORES)
    ns, Qs = plan["ns"], plan["Qs"]
    bigs = plan["bigs"]

    dp = nc.declare_dram_parameter
    W1a_h = dp("w1a", [D, H], f32, isOutput=False)
    W1b_h = dp("w1b", [D, H], f32, isOutput=False)
    b1_h = dp("b1v", [H, 1], f32, isOutput=False)
    w2_h = dp("w2tau", [H, 1], f32, isOutput=False)
    eye_h = dp("eye", [128, 128], f32, isOutput=False)

    big_in = []
    for g in bigs:
        i = g["b"]
        big_in.append({
            "xT": dp(f"xT_big{i}", [D, g["We"]], f32, isOutput=False),
            "xTc": dp(f"xTc_big{i}", [D, g["Q"]], f32, isOutput=False),
            "g": dp(f"g_big{i}", [g["Q"], g["We"]], f32, isOutput=False),
            "tt": dp(f"tt_big{i}", [128, g["rt"], 2], f32, isOutput=False),
        })
    if ns:
        WeS = plan["WeS"]
        xT_s_h = dp("xT_small", [D, WeS], f32, isOutput=False)
        xTc_s_h = dp("xTc_small", [D, Qs], f32, isOutput=False)
        g_s_h = dp("g_small", [Qs, WeS], f32, isOutput=False)
        tt_s_h = dp("tt_small", [128, ns, 2], f32, isOutput=False)

    big_out = []
    for g in bigs:
        i = g["b"]
        big_out.append({
            "perm": dp(f"perm_big{i}", [128, g["rt"], g["W"]], f32, isOutput=True),
            "tt": dp(f"ttout_big{i}", [2, g["W"]], f32, isOutput=True),
        })
    if ns:
        perm_s_h = dp("perm_small", [128, ns, 128], f32, isOutput=True)
        tt_s_out_h = dp("ttout_small", [2, ns * 128], f32, isOutput=True)

    big_sz = sum(g["Q"] * g["We"] for g in bigs)
    small_sz = Qs * plan["WeS"] if ns else 0
    cc_local_b = nc.dram_tensor("cc_local_b", [max(big_sz, 1)], f32)
    cc_gath_b = nc.dram_tensor("cc_gath_b", [NCORES, max(big_sz, 1)], f32,
                               addr_space="Shared")
    if ns:
        cc_local_s = nc.dram_tensor("cc_local_s", [small_sz], f32)
        cc_gath_s = nc.dram_tensor("cc_gath_s", [NCORES, small_sz], f32,
                                   addr_space="Shared")

    with ExitStack() as ctx:
        tc = ctx.enter_context(tile.TileContext(nc))
        const = ctx.enter_context(tc.tile_pool(name="const", bufs=1))
        setup = ctx.enter_context(tc.tile_pool(name="setup", bufs=2))
        rpool = ctx.enter_context(tc.tile_pool(name="relu", bufs=RELU_BUFS))
        spool = ctx.enter_context(tc.tile_pool(name="scratch", bufs=3))
        kpool = ctx.enter_context(tc.tile_pool(name="kmat", bufs=1))
        vpool = ctx.enter_context(tc.tile_pool(name="vecs", bufs=2))
        pp = ctx.enter_context(tc.tile_pool(name="ps", bufs=2, space="PSUM"))
        ppk = ctx.enter_context(tc.tile_pool(name="psk", bufs=2, space="PSUM"))

        # constants
        W1a = const.tile([D, H], f32)
        W1b = const.tile([D, H], f32)
        b1v = const.tile([H, 1], f32)
        eye = const.tile([128, 128], f32)
        w2sb = const.tile([H, 1], f32)
        zw2 = const.tile([H, 129], f32r)      # cols 0..127 zero, col 128 = W2/tau
        onesT = const.tile([1, 128], f32)
        nc.sync.dma_start(W1a[:], W1a_h[:])
        nc.sync.dma_start(W1b[:], W1b_h[:])
        nc.sync.dma_start(b1v[:], b1_h[:])
        nc.sync.dma_start(eye[:], eye_h[:])
        nc.sync.dma_start(w2sb[:], w2_h[:])
        nc.vector.memset(zw2[:, 0:128].bitcast(f32), 0.0)
        nc.vector.tensor_copy(zw2[:, 128:129], w2sb[:])   # f32 -> f32r round
        nc.vector.memset(onesT[:], 1.0)

        # ---------------- phase A: P0 chunks
        def phase_a(xT_h, xTc_h, g_h, Q, W, act_num, act_den, ccbuf, off):
            xT = setup.tile([D, W], f32, tag="xT")
            xTc = setup.tile([D, Q], f32, tag="xTc")
            gsb = setup.tile([Q, W], f32, tag="gsb")
            nc.sync.dma_start(xT[:], xT_h[:])
            nc.sync.dma_start(xTc[:], xTc_h[:])
            nc.sync.dma_start(gsb[:], g_h[:])

            ps_pj = pp.tile([H, W], f32, tag="pjpsum")
            nc.tensor.matmul(ps_pj[:], lhsT=W1b[:], rhs=xT[:], start=True, stop=True)
            pjT = setup.tile([H, W], f32, tag="pjT")
            nc.scalar.copy(pjT[:], ps_pj[:])

            ps_pi = pp.tile([H, Q], f32, tag="pipsum")
            nc.tensor.matmul(ps_pi[:], lhsT=W1a[:], rhs=xTc[:], start=True, stop=True)
            piTb = setup.tile([H, Q], f32, tag="piTb")
            nc.vector.tensor_scalar(piTb[:], ps_pi[:], b1v[:, 0:1], None, op0=OP.add)

            ps_sc = pp.tile([128, W], f32, tag="scpsum")
            for q in reversed(range(Q)):
                rt_ = rpool.tile([H, W], f32r, tag="relu_t")
                if act_num and (q % act_den) < act_num:
                    nc.scalar.activation(rt_[:], pjT[:], AF.Relu,
                                         bias=piTb[:, q:q + 1], scale=1.0)
                else:
                    nc.vector.tensor_scalar(rt_[:], pjT[:], piTb[:, q:q + 1], 0.0,
                                            op0=OP.add, op1=OP.max)
                nc.tensor.matmul(ps_sc[0:q + 1, :],
                                 lhsT=zw2[:, 128 - q:129],
                                 rhs=rt_[:],
                                 start=(q == Q - 1), stop=(q == 0),
                                 skip_group_check=True)
            t = spool.tile([Q, W], f32, tag="p0chunk")
            nc.vector.tensor_tensor(t[:], ps_sc[0:Q, :], gsb[:], op=OP.add)
            nc.scalar.activation(t[:], t[:], AF.Exp, bias=0.0, scale=1.0)
            st = nc.sync.dma_start(
                ccbuf[off:off + Q * W].rearrange("(q w) -> q w", w=W), t[:])
            return st

        # smalls first so their (smaller) collective hides under the big loop
        if ns:
            st_s = phase_a(xT_s_h, xTc_s_h, g_s_h, Qs, plan["WeS"], 0, 1,
                           cc_local_s, 0)
        big_stores = []
        off = 0
        for g, ih in zip(bigs, big_in):
            big_stores.append(phase_a(ih["xT"], ih["xTc"], ih["g"], g["Q"],
                                      g["We"], ACT_NUM, ACT_DEN, cc_local_b, off))
            off += g["Q"] * g["We"]

        # ---------------- allgathers
        if PHASE_LIMIT == "A":
            _set_partial(nc)
            raise _PhaseDone()
        grp = [list(range(NCORES))]
        if ns:
            cc_s = nc.gpsimd.collective_compute(
                "AllGather", OP.bypass, replica_groups=grp,
                ins=[cc_local_s[:]], outs=[cc_gath_s[:]])
            add_dep_helper(cc_s.ins, st_s.ins, sync=True,
                           reason="cc_s waits small chunk store")
        cc_b = None
        if bigs:
            cc_b = nc.gpsimd.collective_compute(
                "AllGather", OP.bypass, replica_groups=grp,
                ins=[cc_local_b[:]], outs=[cc_gath_b[:]])
            for st in big_stores:
                add_dep_helper(cc_b.ins, st.ins, sync=True,
                               reason="cc_b waits big chunk store")

        _ld_engines = [nc.sync, nc.scalar]
        _ld_rr = [0]

        def gathered_load(gath, cc, dst_ap, core, start_el, q0, nrows, W):
            src = gath[core, start_el + q0 * W: start_el + (q0 + nrows) * W]
            eng = _ld_engines[_ld_rr[0] % len(_ld_engines)]
            _ld_rr[0] += 1
            ld = eng.dma_start(dst_ap, src.rearrange("(q w) -> q w", w=W))
            add_dep_helper(ld.ins, cc.ins, sync=True, reason="load waits cc")

        if PHASE_LIMIT == "gather":
            _set_partial(nc)
            raise _PhaseDone()
        # ---------------- assemble K0 matrices (smalls first)
        if ns:
            WeS = plan["WeS"]
            K0S = kpool.tile([128, ns, 128], f32, tag="K0S")
            nc.vector.memset(K0S[:], 0.0)
            for c in range(NCORES):
                k, ci = plan["core_small"][c]
                if c >= sum(sm["alloc"] for sm in plan["smalls"]):
                    continue
                p0 = ci * Qs
                gathered_load(cc_gath_s, cc_s, K0S[p0:p0 + Qs, k, 0:WeS],
                              c, 0, 0, Qs, WeS)
        big_k = []
        off = 0
        for g in bigs:
            Q, W, We, rt, ct = g["Q"], g["W"], g["We"], g["rt"], g["ct"]
            K0 = kpool.tile([128, rt, W], f32, tag=f"K0_{g['b']}")
            nc.vector.memset(K0[:], 0.0)
            for c in range(NCORES):
                q0 = 0
                while q0 < Q:
                    grow = c * Q + q0
                    tile_i, p0 = grow // 128, grow % 128
                    nrows = min(Q - q0, 128 - p0)
                    gathered_load(cc_gath_b, cc_b, K0[p0:p0 + nrows, tile_i, 0:We],
                                  c, off, q0, nrows, We)
                    q0 += nrows
            big_k.append(K0)
            off += Q * We

        if PHASE_LIMIT == "k0":
            _set_partial(nc)
            raise _PhaseDone()
        # ---------------- transposes -> KT0 (smalls first)
        if ns:
            KT0S = kpool.tile([128, ns, 128], f32, tag="KT0S")
            for k in range(ns):
                psT = ppk.tile([128, 128], f32, tag="w")
                nc.tensor.transpose(psT[:], K0S[:, k, :], eye[:])
                nc.vector.tensor_copy(KT0S[:, k, :], psT[:])
        big_kt = []
        for g, K0 in zip(bigs, big_k):
            rt, ct = g["rt"], g["ct"]
            KT0 = kpool.tile([128, ct, rt * 128], f32, tag=f"KT0_{g['b']}")
            for jc in range(ct):
                psT = ppk.tile([128, rt * 128], f32, tag="psT")
                for ic in range(rt):
                    nc.tensor.transpose(psT[:, ic * 128:(ic + 1) * 128],
                                        K0[:, ic, jc * 128:(jc + 1) * 128], eye[:])
                nc.vector.tensor_copy(KT0[:, jc, :], psT[:])
            big_kt.append(KT0)
        if ns:
            KT0S = kpool.tile([128, ns, 128], f32, tag="KT0S")
            for k in range(ns):
                psT = ppk.tile([128, 128], f32, tag="psTS")
                nc.tensor.transpose(psT[:], K0S[:, k, :], eye[:])
                nc.vector.tensor_copy(KT0S[:, k, :], psT[:])

        # ---------------- sinkhorn (scaling-vector form)
        # Reference quirk (f32 absorption at -1e9): after row-normalization the
        # fully-masked rows become exp(0)=1.0, polluting every column sum by
        # exactly (L - s). Divisor recurrence: d_t = K^T u_t + (L-s) * d_{t-1}.
        big_uc = []
        for g in bigs:
            c_t = vpool.tile([128, g["ct"]], f32, tag=f"c_{g['b']}")
            nc.vector.memset(c_t[:], 1.0)
            d_t = vpool.tile([128, g["ct"]], f32, tag=f"d_{g['b']}")
            nc.vector.memset(d_t[:], 1.0)
            big_uc.append({"c": c_t, "d": d_t, "u": None, "pol": float(L - g["s"])})
        if ns:
            cS = vpool.tile([128, ns], f32, tag="cS")
            nc.vector.memset(cS[:], 1.0)
            dS = vpool.tile([128, ns], f32, tag="dS")
            nc.vector.memset(dS[:], 1.0)
            polS = const.tile([128, ns], f32)
            for k, sm in enumerate(plan["smalls"]):
                nc.vector.memset(polS[:, k:k + 1], float(L - sm["s"]))
            uS = None

        for it in range(ITERS):
            for g, K0, KT0, uc in zip(bigs, big_k, big_kt, big_uc):
                rt, ct = g["rt"], g["ct"]
                ps_u = pp.tile([128, rt], f32, tag="ps_u")
                for ic in range(rt):
                    for jc in range(ct):
                        nc.tensor.matmul(ps_u[:, ic:ic + 1],
                                         lhsT=KT0[:, jc, ic * 128:(ic + 1) * 128],
                                         rhs=uc["c"][:, jc:jc + 1],
                                         start=(jc == 0), stop=(jc == ct - 1))
                tmp = spool.tile([128, rt], f32, tag="tmp_u")
                nc.vector.tensor_scalar(tmp[:], ps_u[:], EPS, None, op0=OP.add)
                u_t = vpool.tile([128, rt], f32, tag=f"u_{g['b']}")
                nc.vector.reciprocal(u_t[:], tmp[:])
                uc["u"] = u_t

                ps_c = pp.tile([128, ct], f32, tag="ps_c")
                for jc in range(ct):
                    for ic in range(rt):
                        nc.tensor.matmul(ps_c[:, jc:jc + 1],
                                         lhsT=K0[:, ic, jc * 128:(jc + 1) * 128],
                                         rhs=u_t[:, ic:ic + 1],
                                         start=(ic == 0), stop=(ic == rt - 1))
                dscale = spool.tile([128, ct], f32, tag="tmp_c")
                nc.vector.tensor_scalar(dscale[:], uc["d"][:], uc["pol"], None,
                                        op0=OP.mult)
                d_t = vpool.tile([128, ct], f32, tag=f"d_{g['b']}")
                nc.vector.tensor_tensor(d_t[:], ps_c[:], dscale[:], op=OP.add)
                uc["d"] = d_t
                c_t = vpool.tile([128, ct], f32, tag=f"c_{g['b']}")
                nc.vector.reciprocal(c_t[:], d_t[:])
                uc["c"] = c_t

            if ns:
                ps_uS = pp.tile([128, ns], f32, tag="ps_uS")
                for k in range(ns):
                    nc.tensor.matmul(ps_uS[:, k:k + 1], lhsT=KT0S[:, k, :],
                                     rhs=cS[:, k:k + 1], start=True, stop=True)
                tmpS = spool.tile([128, ns], f32, tag="tmp_uS")
                nc.vector.tensor_scalar(tmpS[:], ps_uS[:], EPS, None, op0=OP.add)
                uS = vpool.tile([128, ns], f32, tag="uS")
                nc.vector.reciprocal(uS[:], tmpS[:])

                ps_cS = pp.tile([128, ns], f32, tag="ps_cS")
                for k in range(ns):
                    nc.tensor.matmul(ps_cS[:, k:k + 1], lhsT=K0S[:, k, :],
                                     rhs=uS[:, k:k + 1], start=True, stop=True)
                dscaleS = spool.tile([128, ns], f32, tag="tmp_cS")
                nc.vector.tensor_tensor(dscaleS[:], dS[:], polS[:], op=OP.mult)
                dS = vpool.tile([128, ns], f32, tag="dS")
                nc.vector.tensor_tensor(dS[:], ps_cS[:], dscaleS[:], op=OP.add)
                cS = vpool.tile([128, ns], f32, tag="cS")
                nc.vector.reciprocal(cS[:], dS[:])

        if PHASE_LIMIT == "sink":
            _set_partial(nc)
            raise _PhaseDone()
        # ---------------- final perm + einsums
        def finalize(K0, u_t, c_t, rt, ct, W, tt_in, perm_out, tt_out):
            ps_crow = pp.tile([1, W], f32, tag="ps_crow")
            for jc in range(ct):
                nc.tensor.transpose(ps_crow[0:1, jc * 128:(jc + 1) * 128],
                                    c_t[:, jc:jc + 1], eye[:])
            crow = spool.tile([1, W], f32, tag="crow")
            nc.vector.tensor_copy(crow[:], ps_crow[:])
            ps_bc = pp.tile([128, W], f32, tag="ps_bc")
            nc.tensor.matmul(ps_bc[:], lhsT=onesT[:], rhs=crow[:], start=True,
                             stop=True)
            for ic in range(rt):
                nc.vector.tensor_scalar(K0[:, ic, :], K0[:, ic, :],
                                        u_t[:, ic:ic + 1], None, op0=OP.mult)
                nc.vector.tensor_tensor(K0[:, ic, :], K0[:, ic, :], ps_bc[:],
                                        op=OP.mult)
            ps_tt = pp.tile([2, W], f32, tag="ps_tt")
            for ic in range(rt):
                nc.tensor.matmul(ps_tt[:], lhsT=tt_in[:, ic, :], rhs=K0[:, ic, :],
                                 start=(ic == 0), stop=(ic == rt - 1))
            ttsb = spool.tile([2, W], f32, tag="ttsb")
            nc.vector.tensor_copy(ttsb[:], ps_tt[:])
            nc.sync.dma_start(tt_out[:], ttsb[:])
            nc.sync.dma_start(perm_out[:], K0[:])

        for g, K0, uc, ih, oh in zip(bigs, big_k, big_uc, big_in, big_out):
            tt_in = setup.tile([128, g["rt"], 2], f32, tag="ttbig")
            nc.sync.dma_start(tt_in[:], ih["tt"][:])
            finalize(K0, uc["u"], uc["c"], g["rt"], g["ct"], g["W"],
                     tt_in, oh["perm"], oh["tt"])
        if ns:
            tt_in = setup.tile([128, ns, 2], f32, tag="ttsmall")
            nc.sync.dma_start(tt_in[:], tt_s_h[:])
            ps_crow = pp.tile([1, ns * 128], f32, tag="ps_crowS")
            for k in range(ns):
                nc.tensor.transpose(ps_crow[0:1, k * 128:(k + 1) * 128],
                                    cS[:, k:k + 1], eye[:])
            crow = spool.tile([1, ns * 128], f32, tag="crowS")
            nc.vector.tensor_copy(crow[:], ps_crow[:])
            ps_bc = pp.tile([128, ns, 128], f32, tag="ps_bcS")
            nc.tensor.matmul(ps_bc[:].rearrange("p k w -> p (k w)"), lhsT=onesT[:],
                             rhs=crow[:], start=True, stop=True)
            for k in range(ns):
                nc.vector.tensor_scalar(K0S[:, k, :], K0S[:, k, :], uS[:, k:k + 1],
                                        None, op0=OP.mult)
            nc.vector.tensor_tensor(K0S[:], K0S[:], ps_bc[:], op=OP.mult)
            ps_tt = pp.tile([2, ns * 128], f32, tag="ps_ttS")
            for k in range(ns):
                nc.tensor.matmul(ps_tt[:, k * 128:(k + 1) * 128],
                                 lhsT=tt_in[:, k, :], rhs=K0S[:, k, :],
                                 start=True, stop=True)
            ttsb = spool.tile([2, ns * 128], f32, tag="ttsbS")
            nc.vector.tensor_copy(ttsb[:], ps_tt[:])
            nc.sync.dma_start(tt_s_out_h[:], ttsb[:])
            nc.sync.dma_start(perm_s_h[:], K0S[:])

    out_names = [f"perm_big{g['b']}" for g in bigs] + \
                [f"ttout_big{g['b']}" for g in bigs]
    if ns:
        out_names += ["perm_small", "ttout_small"]
    return nc, out_names


# ---------------------------------------------------------------- host side
_compiled = {}


def kernel(event_time, event_type, clean_enc_out, seq_lens, gumbel_noise,
           W1, b1, W2, b2):
    _install_wait_fix()
    event_time = np.asarray(event_time, dtype=np.float32)
    event_type_f = np.asarray(event_type).astype(np.float32)
    x = np.asarray(clean_enc_out, dtype=np.float32)
    s_arr = [int(v) for v in np.asarray(seq_lens)]
    g_all = np.asarray(gumbel_noise, dtype=np.float32)
    W1 = np.asarray(W1, dtype=np.float32)
    b1 = np.asarray(b1, dtype=np.float32)
    W2 = np.asarray(W2, dtype=np.float32)
    b2 = np.asarray(b2, dtype=np.float32)

    plan = _plan(s_arr)
    key = tuple(s_arr)
    if key not in _compiled:
        _compiled[key] = _build(plan)
    nc, out_names = _compiled[key]

    ns, Qs = plan["ns"], plan["Qs"]
    eye = np.eye(128, dtype=np.float32)
    shared = {
        "w1a": np.ascontiguousarray(W1[:D]),
        "w1b": np.ascontiguousarray(W1[D:]),
        "b1v": np.ascontiguousarray(b1[:, None]),
        "w2tau": np.ascontiguousarray(W2[:, 0:1] / TAU),
        "eye": eye,
    }
    gb2tau = (b2[0] / TAU).astype(np.float32)

    def gum_block(b, rows, W, s):
        # (gumbel + b2)/tau for valid (row<s, col<s); NEG_FILL elsewhere
        out = np.full((len(rows), W), NEG_FILL, dtype=np.float32)
        for q, r in enumerate(rows):
            if r < s:
                out[q, :s] = g_all[b, r, :s] / TAU + gb2tau
        return out

    big_static = []
    for g in plan["bigs"]:
        b, s, Q, W, rt = g["b"], g["s"], g["Q"], g["We"], g["rt"]
        xT = np.zeros((D, W), dtype=np.float32)
        xT[:, :s] = x[b, :s].T
        tt = np.zeros((128, rt, 2), dtype=np.float32)
        for r in range(s):
            tt[r % 128, r // 128, 0] = event_type_f[b, r]
            tt[r % 128, r // 128, 1] = event_time[b, r]
        big_static.append({"xT": xT, "tt": tt})

    if ns:
        tts = np.zeros((128, ns, 2), dtype=np.float32)
        for k, sm in enumerate(plan["smalls"]):
            b, s = sm["b"], sm["s"]
            tts[:s, k, 0] = event_type_f[b, :s]
            tts[:s, k, 1] = event_time[b, :s]

    in_maps = []
    for c in range(NCORES):
        m = dict(shared)
        for g, st in zip(plan["bigs"], big_static):
            b, s, Q, W = g["b"], g["s"], g["Q"], g["We"]
            i = b
            rows = list(range(c * Q, (c + 1) * Q))
            xTc = np.zeros((D, Q), dtype=np.float32)
            for q, r in enumerate(rows):
                if r < s:
                    xTc[:, q] = x[b, r]
            m[f"xT_big{i}"] = st["xT"]
            m[f"xTc_big{i}"] = xTc
            m[f"g_big{i}"] = gum_block(b, rows, W, s)
            m[f"tt_big{i}"] = st["tt"]
        if ns:
            k, ci = plan["core_small"][c]
            sm = plan["smalls"][k]
            b, s = sm["b"], sm["s"]
            WeS = plan["WeS"]
            xTs = np.zeros((D, WeS), dtype=np.float32)
            xTs[:, :s] = x[b, :s].T
            rows = list(range(ci * Qs, (ci + 1) * Qs))
            xTcs = np.zeros((D, Qs), dtype=np.float32)
            for q, r in enumerate(rows):
                if r < s:
                    xTcs[:, q] = x[b, r]
            m["xT_small"] = xTs
            m["xTc_small"] = xTcs
            m["g_small"] = gum_block(b, rows, WeS, s)
            m["tt_small"] = tts
        in_maps.append(m)

    if os.environ.get("TRNK_BACKEND") == "sim":
        from concourse.bass_interp import MultiCoreSim
        sim = MultiCoreSim(nc, NCORES)
        for c, m in enumerate(in_maps):
            for k, v in m.items():
                sim.cores[c].tensor(k)[:] = v
        sim.simulate()
        r0 = {name: np.array(sim.cores[0].tensor(name)) for name in out_names}
    else:
        res = run_bass_kernel_spmd(nc, in_maps, core_ids=list(range(NCORES)),
                                   trace=bool(os.environ.get("TRNK_TRACE")))
        if res.exec_time_ns is not None:
            kernel.last_exec_time_ns = res.exec_time_ns
            kernel.last_profile = res
        r0 = res.results[0]

    perm = np.zeros((B, L, L), dtype=np.float32)
    tp = np.zeros((B, L), dtype=np.float32)
    tm = np.zeros((B, L), dtype=np.float32)
    for g in plan["bigs"]:
        b, s, rt, W = g["b"], g["s"], g["rt"], g["W"]
        blk = r0[f"perm_big{b}"].transpose(1, 0, 2).reshape(rt * 128, W)
        perm[b, :s, :s] = blk[:s, :s]
        tp[b, :s] = r0[f"ttout_big{b}"][0, :s]
        tm[b, :s] = r0[f"ttout_big{b}"][1, :s]
    if ns:
        for k, sm in enumerate(plan["smalls"]):
            b, s = sm["b"], sm["s"]
            perm[b, :s, :s] = r0["perm_small"][:s, k, :s]
            tp[b, :s] = r0["ttout_small"][0, k * 128:k * 128 + s]
            tm[b, :s] = r0["ttout_small"][1, k * 128:k * 128 + s]
    return tp, tm, perm


kernel.last_exec_time_ns = None
kernel.last_profile = None


# revision 35
# speedup vs baseline: 1.0320x; 1.0320x over previous
"""Trainium2 Bass kernel for nn_AdversarialGenerator (gumbel-sinkhorn permutation).

Contract: kernel(**inputs) takes FULL numpy inputs, returns
(types_permed [B,L] f32, times_permed [B,L] f32, perm [B,L,L] f32).

Strategy (8 NeuronCores, one SPMD launch):
  - Specializes the program on the runtime seq_lens values: only the valid
    s_b x s_b block of each batch is computed; everything else is exactly 0
    (host pads gumbel with -1e30 so exp() kills padding).
  - Phase A: scores + exp, data-parallel over rows across all 8 cores.
    relu rows [H=128, W] on DVE/ACT; the W2-dot runs on the PE via a
    sliding-window masked-weight matrix so score rows accumulate row-major
    in PSUM (f32r for 1 cycle/row).
  - AllGather of the exp(score+gumbel) chunks (ordered with add_dep_helper).
  - Phase B: Sinkhorn in scaling-vector form r=1/(Kc), c=1/(K^T r) -- all
    chunked PE matmuls with [128,k] vector layouts and tiny DVE reciprocals.
    Replicated on every core (cheaper than per-iteration collectives).
  - Phase C: perm = diag(r) K diag(c); type/time einsums on PE; compact
    DMA outputs; host pastes into zeros.
"""
import os
import sys

sys.path.insert(0, "/opt/trn_rl_repo")

from contextlib import ExitStack

import numpy as np
import orjson

import concourse.bass as bass
import concourse.tile as tile
from concourse import mybir
from concourse.bass_utils import run_bass_kernel_spmd
from concourse.tile_rust import add_dep_helper

f32 = mybir.dt.float32
f32r = mybir.dt.float32r
AF = mybir.ActivationFunctionType
OP = mybir.AluOpType

B, L, D, H = 4, 512, 64, 128
TAU = 0.5
ITERS = 10
NCORES = 8
ACT_NUM = 5
ACT_DEN = 11
RELU_BUFS = 8
PHASE_LIMIT = "all"


class _PhaseDone(Exception):
    pass


_build_partial_result = None


def _set_partial(nc):
    global _build_partial_result
    _build_partial_result = (nc, [])

NEG_FILL = -1e30
EPS = 1e-30


# ---------------------------------------------------------------- wait fix
# This container's walrus accepts at most ONE sync wait per instruction.
# Tile attaches several; split the excess onto EventSemaphore carriers
# inserted right before the offender (same engine => same order).
def _legalize_bir_waits(bir: dict, max_waits: int = 1) -> int:
    n = 0
    for func in bir.get("functions", []):
        for bb in func.get("blocks", []):
            out = []
            for ins in bb.get("instructions", []):
                si = ins.get("sync_info")
                waits = (si or {}).get("on_wait") or []
                if len(waits) > max_waits:
                    excess, keep = waits[:-max_waits], waits[-max_waits:]
                    for k, w in enumerate(excess):
                        out.append({
                            "name": f"{ins['name']}_xw{k}",
                            "opcode": "EventSemaphore",
                            "engine": ins["engine"],
                            "ins": [],
                            "outs": [],
                            "sync_info": {"on_wait": [w], "on_update": []},
                            "debug": ins.get("debug"),
                        })
                        n += 1
                    si["on_wait"] = keep
                out.append(ins)
            bb["instructions"] = out
    return n


_patched = False


def _install_wait_fix():
    global _patched
    if _patched:
        return
    _patched = True
    import concourse.bass_utils as bu
    import concourse.bass2jax as b2j

    orig = bu.compile_bir_kernel

    def patched(bir_json, tmpdir, neff_name="file.neff"):
        bir = orjson.loads(bir_json)
        if _legalize_bir_waits(bir):
            bir_json = orjson.dumps(bir)
        return orig(bir_json, tmpdir, neff_name=neff_name)

    bu.compile_bir_kernel = patched
    b2j.compile_bir_kernel = patched


# ---------------------------------------------------------------- layout plan
def _plan(s_list):
    cdiv = lambda a, b: (a + b - 1) // b
    bigs, smalls = [], []
    for b in range(len(s_list)):
        (bigs if s_list[b] > 128 else smalls).append(b)

    plan = {"bigs": [], "smalls": [], "s": list(s_list)}
    for b in bigs:
        s = s_list[b]
        Q = cdiv(s, NCORES)          # rows per core
        R = NCORES * Q               # gathered rows (>= s)
        rt = cdiv(R, 128)            # row tiles
        ct = cdiv(s, 128)            # col tiles
        plan["bigs"].append({"b": b, "s": s, "Q": Q, "R": R, "rt": rt,
                             "ct": ct, "W": ct * 128, "We": 2 * cdiv(s, 2)})
    ns = len(smalls)
    if ns:
        best = None
        import itertools
        for alloc in itertools.product(range(1, NCORES + 1), repeat=ns):
            if sum(alloc) > NCORES:
                continue
            if any(a * cdiv(s_list[b], a) > 128 for a, b in zip(alloc, smalls)):
                continue
            qs = max(cdiv(s_list[b], a) for a, b in zip(alloc, smalls))
            key = (qs, sum(alloc))
            if best is None or key < best[0]:
                best = (key, alloc)
        alloc = list(best[1])
        Qs = max(cdiv(s_list[b], a) for a, b in zip(alloc, smalls))
        # core -> (small index k, chunk index ci); unassigned cores mirror k=0,ci=0
        core_small = [(0, 0)] * NCORES
        cidx = 0
        for k, (a, b) in enumerate(zip(alloc, smalls)):
            for ci in range(a):
                core_small[cidx] = (k, ci)
                cidx += 1
        plan["smalls"] = [{"b": b, "s": s_list[b], "alloc": a}
                          for a, b in zip(alloc, smalls)]
        plan["Qs"] = Qs
        plan["core_small"] = core_small
        plan["ns"] = ns
        plan["WeS"] = 2 * cdiv(max(s_list[b] for b in smalls), 2)
    else:
        plan["WeS"] = 0
        plan["Qs"] = 0
        plan["ns"] = 0
        plan["core_small"] = [(0, 0)] * NCORES
    return plan


# ---------------------------------------------------------------- builder
def _build(plan):
    try:
        return _build_inner(plan)
    except _PhaseDone:
        return _build_partial_result


def _build_inner(plan):
    global _build_partial_result
    nc = bass.Bass(num_devices=NCORES)
    ns, Qs = plan["ns"], plan["Qs"]
    bigs = plan["bigs"]

    dp = nc.declare_dram_parameter
    W1a_h = dp("w1a", [D, H], f32, isOutput=False)
    W1b_h = dp("w1b", [D, H], f32, isOutput=False)
    b1_h = dp("b1v", [H, 1], f32, isOutput=False)
    w2_h = dp("w2tau", [H, 1], f32, isOutput=False)
    eye_h = dp("eye", [128, 128], f32, isOutput=False)

    big_in = []
    for g in bigs:
        i = g["b"]
        big_in.append({
            "xT": dp(f"xT_big{i}", [D, g["We"]], f32, isOutput=False),
            "xTc": dp(f"xTc_big{i}", [D, g["Q"]], f32, isOutput=False),
            "g": dp(f"g_big{i}", [g["Q"], g["We"]], f32, isOutput=False),
            "tt": dp(f"tt_big{i}", [128, g["rt"], 2], f32, isOutput=False),
        })
    if ns:
        WeS = plan["WeS"]
        xT_s_h = dp("xT_small", [D, WeS], f32, isOutput=False)
        xTc_s_h = dp("xTc_small", [D, Qs], f32, isOutput=False)
        g_s_h = dp("g_small", [Qs, WeS], f32, isOutput=False)
        tt_s_h = dp("tt_small", [128, ns, 2], f32, isOutput=False)

    big_out = []
    for g in bigs:
        i = g["b"]
        big_out.append({
            "perm": dp(f"perm_big{i}", [128, g["rt"], g["W"]], f32, isOutput=True),
            "tt": dp(f"ttout_big{i}", [2, g["W"]], f32, isOutput=True),
        })
    if ns:
        perm_s_h = dp("perm_small", [128, ns, 128], f32, isOutput=True)
        tt_s_out_h = dp("ttout_small", [2, ns * 128], f32, isOutput=True)

    big_sz = sum(g["Q"] * g["We"] for g in bigs)
    small_sz = Qs * plan["WeS"] if ns else 0
    cc_local_b = nc.dram_tensor("cc_local_b", [max(big_sz, 1)], f32)
    cc_gath_b = nc.dram_tensor("cc_gath_b", [NCORES, max(big_sz, 1)], f32,
                               addr_space="Shared")
    if ns:
        cc_local_s = nc.dram_tensor("cc_local_s", [small_sz], f32)
        cc_gath_s = nc.dram_tensor("cc_gath_s", [NCORES, small_sz], f32,
                                   addr_space="Shared")

    with ExitStack() as ctx:
        tc = ctx.enter_context(tile.TileContext(nc))
        const = ctx.enter_context(tc.tile_pool(name="const", bufs=1))
        setup = ctx.enter_context(tc.tile_pool(name="setup", bufs=2))
        rpool = ctx.enter_context(tc.tile_pool(name="relu", bufs=RELU_BUFS))
        spool = ctx.enter_context(tc.tile_pool(name="scratch", bufs=3))
        kpool = ctx.enter_context(tc.tile_pool(name="kmat", bufs=1))
        vpool = ctx.enter_context(tc.tile_pool(name="vecs", bufs=2))
        pp = ctx.enter_context(tc.tile_pool(name="ps", bufs=2, space="PSUM"))
        ppk = ctx.enter_context(tc.tile_pool(name="psk", bufs=2, space="PSUM"))

        # constants
        W1a = const.tile([D, H], f32)
        W1b = const.tile([D, H], f32)
        b1v = const.tile([H, 1], f32)
        eye = const.tile([128, 128], f32)
        w2sb = const.tile([H, 1], f32)
        zw2 = const.tile([H, 129], f32r)      # cols 0..127 zero, col 128 = W2/tau
        onesT = const.tile([1, 128], f32)
        nc.sync.dma_start(W1a[:], W1a_h[:])
        nc.sync.dma_start(W1b[:], W1b_h[:])
        nc.sync.dma_start(b1v[:], b1_h[:])
        nc.sync.dma_start(eye[:], eye_h[:])
        nc.sync.dma_start(w2sb[:], w2_h[:])
        nc.vector.memset(zw2[:, 0:128].bitcast(f32), 0.0)
        nc.vector.tensor_copy(zw2[:, 128:129], w2sb[:])   # f32 -> f32r round
        nc.vector.memset(onesT[:], 1.0)

        # ---------------- phase A: P0 chunks
        def phase_a(xT_h, xTc_h, g_h, Q, W, act_num, act_den, ccbuf, off):
            xT = setup.tile([D, W], f32, tag="xT")
            xTc = setup.tile([D, Q], f32, tag="xTc")
            gsb = setup.tile([Q, W], f32, tag="gsb")
            nc.sync.dma_start(xT[:], xT_h[:])
            nc.sync.dma_start(xTc[:], xTc_h[:])
            nc.sync.dma_start(gsb[:], g_h[:])

            ps_pj = pp.tile([H, W], f32, tag="pjpsum")
            nc.tensor.matmul(ps_pj[:], lhsT=W1b[:], rhs=xT[:], start=True, stop=True)
            pjT = setup.tile([H, W], f32, tag="pjT")
            nc.scalar.copy(pjT[:], ps_pj[:])

            ps_pi = pp.tile([H, Q], f32, tag="pipsum")
            nc.tensor.matmul(ps_pi[:], lhsT=W1a[:], rhs=xTc[:], start=True, stop=True)
            piTb = setup.tile([H, Q], f32, tag="piTb")
            nc.vector.tensor_scalar(piTb[:], ps_pi[:], b1v[:, 0:1], None, op0=OP.add)

            ps_sc = pp.tile([128, W], f32, tag="scpsum")
            for q in reversed(range(Q)):
                rt_ = rpool.tile([H, W], f32r, tag="relu_t")
                if act_num and (q % act_den) < act_num:
                    nc.scalar.activation(rt_[:], pjT[:], AF.Relu,
                                         bias=piTb[:, q:q + 1], scale=1.0)
                else:
                    nc.vector.tensor_scalar(rt_[:], pjT[:], piTb[:, q:q + 1], 0.0,
                                            op0=OP.add, op1=OP.max)
                nc.tensor.matmul(ps_sc[0:q + 1, :],
                                 lhsT=zw2[:, 128 - q:129],
                                 rhs=rt_[:],
                                 start=(q == Q - 1), stop=(q == 0),
                                 skip_group_check=True)
            t = spool.tile([Q, W], f32, tag="p0chunk")
            nc.vector.tensor_tensor(t[:], ps_sc[0:Q, :], gsb[:], op=OP.add)
            nc.scalar.activation(t[:], t[:], AF.Exp, bias=0.0, scale=1.0)
            st = nc.sync.dma_start(
                ccbuf[off:off + Q * W].rearrange("(q w) -> q w", w=W), t[:])
            return st

        # smalls first so their (smaller) collective hides under the big loop
        if ns:
            st_s = phase_a(xT_s_h, xTc_s_h, g_s_h, Qs, plan["WeS"], 0, 1,
                           cc_local_s, 0)
        big_stores = []
        off = 0
        for g, ih in zip(bigs, big_in):
            big_stores.append(phase_a(ih["xT"], ih["xTc"], ih["g"], g["Q"],
                                      g["We"], ACT_NUM, ACT_DEN, cc_local_b, off))
            off += g["Q"] * g["We"]

        # ---------------- allgathers
        if PHASE_LIMIT == "A":
            _set_partial(nc)
            raise _PhaseDone()
        grp = [list(range(NCORES))]
        if ns:
            cc_s = nc.gpsimd.collective_compute(
                "AllGather", OP.bypass, replica_groups=grp,
                ins=[cc_local_s[:]], outs=[cc_gath_s[:]])
            add_dep_helper(cc_s.ins, st_s.ins, sync=True,
                           reason="cc_s waits small chunk store")
        cc_b = None
        if bigs:
            cc_b = nc.gpsimd.collective_compute(
                "AllGather", OP.bypass, replica_groups=grp,
                ins=[cc_local_b[:]], outs=[cc_gath_b[:]])
            for st in big_stores:
                add_dep_helper(cc_b.ins, st.ins, sync=True,
                               reason="cc_b waits big chunk store")

        _ld_engines = [nc.sync, nc.scalar]
        _ld_big_engines = [nc.sync, nc.scalar, nc.gpsimd]
        _ld_rr = [0]

        def gathered_load(gath, cc, dst_ap, core, start_el, q0, nrows, W,
                          big=False):
            src = gath[core, start_el + q0 * W: start_el + (q0 + nrows) * W]
            engines = _ld_big_engines if big else _ld_engines
            eng = engines[_ld_rr[0] % len(engines)]
            _ld_rr[0] += 1
            ld = eng.dma_start(dst_ap, src.rearrange("(q w) -> q w", w=W))
            add_dep_helper(ld.ins, cc.ins, sync=True, reason="load waits cc")

        if PHASE_LIMIT == "gather":
            _set_partial(nc)
            raise _PhaseDone()
        # ---------------- assemble K0 matrices (smalls first)
        if ns:
            WeS = plan["WeS"]
            K0S = kpool.tile([128, ns, 128], f32, tag="K0S")
            nc.vector.memset(K0S[:], 0.0)
            for c in range(NCORES):
                k, ci = plan["core_small"][c]
                if c >= sum(sm["alloc"] for sm in plan["smalls"]):
                    continue
                p0 = ci * Qs
                gathered_load(cc_gath_s, cc_s, K0S[p0:p0 + Qs, k, 0:WeS],
                              c, 0, 0, Qs, WeS)
        big_k = []
        off = 0
        for g in bigs:
            Q, W, We, rt, ct = g["Q"], g["W"], g["We"], g["rt"], g["ct"]
            K0 = kpool.tile([128, rt, W], f32, tag=f"K0_{g['b']}")
            nc.vector.memset(K0[:], 0.0)
            for c in range(NCORES):
                q0 = 0
                while q0 < Q:
                    grow = c * Q + q0
                    tile_i, p0 = grow // 128, grow % 128
                    nrows = min(Q - q0, 128 - p0)
                    gathered_load(cc_gath_b, cc_b, K0[p0:p0 + nrows, tile_i, 0:We],
                                  c, off, q0, nrows, We, big=True)
                    q0 += nrows
            big_k.append(K0)
            off += Q * We

        if PHASE_LIMIT == "k0":
            _set_partial(nc)
            raise _PhaseDone()
        # ---------------- transposes -> KT0 (smalls first)
        if ns:
            KT0S = kpool.tile([128, ns, 128], f32, tag="KT0S")
            for k in range(ns):
                psT = ppk.tile([128, 128], f32, tag="w")
                nc.tensor.transpose(psT[:], K0S[:, k, :], eye[:])
                nc.vector.tensor_copy(KT0S[:, k, :], psT[:])
        big_kt = []
        for g, K0 in zip(bigs, big_k):
            rt, ct = g["rt"], g["ct"]
            KT0 = kpool.tile([128, ct, rt * 128], f32, tag=f"KT0_{g['b']}")
            for jc in range(ct):
                psT = ppk.tile([128, rt * 128], f32, tag="psT")
                for ic in range(rt):
                    nc.tensor.transpose(psT[:, ic * 128:(ic + 1) * 128],
                                        K0[:, ic, jc * 128:(jc + 1) * 128], eye[:])
                nc.vector.tensor_copy(KT0[:, jc, :], psT[:])
            big_kt.append(KT0)
        if ns:
            KT0S = kpool.tile([128, ns, 128], f32, tag="KT0S")
            for k in range(ns):
                psT = ppk.tile([128, 128], f32, tag="psTS")
                nc.tensor.transpose(psT[:], K0S[:, k, :], eye[:])
                nc.vector.tensor_copy(KT0S[:, k, :], psT[:])

        # ---------------- sinkhorn (scaling-vector form)
        # Reference quirk (f32 absorption at -1e9): after row-normalization the
        # fully-masked rows become exp(0)=1.0, polluting every column sum by
        # exactly (L - s). Divisor recurrence: d_t = K^T u_t + (L-s) * d_{t-1}.
        big_uc = []
        for g in bigs:
            c_t = vpool.tile([128, g["ct"]], f32, tag=f"c_{g['b']}")
            nc.vector.memset(c_t[:], 1.0)
            d_t = vpool.tile([128, g["ct"]], f32, tag=f"d_{g['b']}")
            nc.vector.memset(d_t[:], 1.0)
            big_uc.append({"c": c_t, "d": d_t, "u": None, "pol": float(L - g["s"])})
        if ns:
            cS = vpool.tile([128, ns], f32, tag="cS")
            nc.vector.memset(cS[:], 1.0)
            dS = vpool.tile([128, ns], f32, tag="dS")
            nc.vector.memset(dS[:], 1.0)
            polS = const.tile([128, ns], f32)
            for k, sm in enumerate(plan["smalls"]):
                nc.vector.memset(polS[:, k:k + 1], float(L - sm["s"]))
            uS = None

        for it in range(ITERS):
            for g, K0, KT0, uc in zip(bigs, big_k, big_kt, big_uc):
                rt, ct = g["rt"], g["ct"]
                ps_u = pp.tile([128, rt], f32, tag="ps_u")
                for ic in range(rt):
                    for jc in range(ct):
                        nc.tensor.matmul(ps_u[:, ic:ic + 1],
                                         lhsT=KT0[:, jc, ic * 128:(ic + 1) * 128],
                                         rhs=uc["c"][:, jc:jc + 1],
                                         start=(jc == 0), stop=(jc == ct - 1))
                tmp = spool.tile([128, rt], f32, tag="tmp_u")
                nc.vector.tensor_scalar(tmp[:], ps_u[:], EPS, None, op0=OP.add)
                u_t = vpool.tile([128, rt], f32, tag=f"u_{g['b']}")
                nc.vector.reciprocal(u_t[:], tmp[:])
                uc["u"] = u_t

                ps_c = pp.tile([128, ct], f32, tag="ps_c")
                for jc in range(ct):
                    for ic in range(rt):
                        nc.tensor.matmul(ps_c[:, jc:jc + 1],
                                         lhsT=K0[:, ic, jc * 128:(jc + 1) * 128],
                                         rhs=u_t[:, ic:ic + 1],
                                         start=(ic == 0), stop=(ic == rt - 1))
                dscale = spool.tile([128, ct], f32, tag="tmp_c")
                nc.vector.tensor_scalar(dscale[:], uc["d"][:], uc["pol"], None,
                                        op0=OP.mult)
                d_t = vpool.tile([128, ct], f32, tag=f"d_{g['b']}")
                nc.vector.tensor_tensor(d_t[:], ps_c[:], dscale[:], op=OP.add)
                uc["d"] = d_t
                c_t = vpool.tile([128, ct], f32, tag=f"c_{g['b']}")
                nc.vector.reciprocal(c_t[:], d_t[:])
                uc["c"] = c_t

            if ns:
                ps_uS = pp.tile([128, ns], f32, tag="ps_uS")
                for k in range(ns):
                    nc.tensor.matmul(ps_uS[:, k:k + 1], lhsT=KT0S[:, k, :],
                                     rhs=cS[:, k:k + 1], start=True, stop=True)
                tmpS = spool.tile([128, ns], f32, tag="tmp_uS")
                nc.vector.tensor_scalar(tmpS[:], ps_uS[:], EPS, None, op0=OP.add)
                uS = vpool.tile([128, ns], f32, tag="uS")
                nc.vector.reciprocal(uS[:], tmpS[:])

                ps_cS = pp.tile([128, ns], f32, tag="ps_cS")
                for k in range(ns):
                    nc.tensor.matmul(ps_cS[:, k:k + 1], lhsT=K0S[:, k, :],
                                     rhs=uS[:, k:k + 1], start=True, stop=True)
                dscaleS = spool.tile([128, ns], f32, tag="tmp_cS")
                nc.vector.tensor_tensor(dscaleS[:], dS[:], polS[:], op=OP.mult)
                dS = vpool.tile([128, ns], f32, tag="dS")
                nc.vector.tensor_tensor(dS[:], ps_cS[:], dscaleS[:], op=OP.add)
                cS = vpool.tile([128, ns], f32, tag="cS")
                nc.vector.reciprocal(cS[:], dS[:])

        if PHASE_LIMIT == "sink":
            _set_partial(nc)
            raise _PhaseDone()
        # ---------------- final perm + einsums
        def finalize(K0, u_t, c_t, rt, ct, W, tt_in, perm_out, tt_out):
            ps_crow = pp.tile([1, W], f32, tag="ps_crow")
            for jc in range(ct):
                nc.tensor.transpose(ps_crow[0:1, jc * 128:(jc + 1) * 128],
                                    c_t[:, jc:jc + 1], eye[:])
            crow = spool.tile([1, W], f32, tag="crow")
            nc.vector.tensor_copy(crow[:], ps_crow[:])
            ps_bc = pp.tile([128, W], f32, tag="ps_bc")
            nc.tensor.matmul(ps_bc[:], lhsT=onesT[:], rhs=crow[:], start=True,
                             stop=True)
            for ic in range(rt):
                nc.vector.tensor_scalar(K0[:, ic, :], K0[:, ic, :],
                                        u_t[:, ic:ic + 1], None, op0=OP.mult)
                nc.vector.tensor_tensor(K0[:, ic, :], K0[:, ic, :], ps_bc[:],
                                        op=OP.mult)
            ps_tt = pp.tile([2, W], f32, tag="ps_tt")
            for ic in range(rt):
                nc.tensor.matmul(ps_tt[:], lhsT=tt_in[:, ic, :], rhs=K0[:, ic, :],
                                 start=(ic == 0), stop=(ic == rt - 1))
            ttsb = spool.tile([2, W], f32, tag="ttsb")
            nc.vector.tensor_copy(ttsb[:], ps_tt[:])
            nc.sync.dma_start(tt_out[:], ttsb[:])
            nc.sync.dma_start(perm_out[:], K0[:])

        for g, K0, uc, ih, oh in zip(bigs, big_k, big_uc, big_in, big_out):
            tt_in = setup.tile([128, g["rt"], 2], f32, tag="ttbig")
            nc.sync.dma_start(tt_in[:], ih["tt"][:])
            finalize(K0, uc["u"], uc["c"], g["rt"], g["ct"], g["W"],
                     tt_in, oh["perm"], oh["tt"])
        if ns:
            tt_in = setup.tile([128, ns, 2], f32, tag="ttsmall")
            nc.sync.dma_start(tt_in[:], tt_s_h[:])
            ps_crow = pp.tile([1, ns * 128], f32, tag="ps_crowS")
            for k in range(ns):
                nc.tensor.transpose(ps_crow[0:1, k * 128:(k + 1) * 128],
                                    cS[:, k:k + 1], eye[:])
            crow = spool.tile([1, ns * 128], f32, tag="crowS")
            nc.vector.tensor_copy(crow[:], ps_crow[:])
            ps_bc = pp.tile([128, ns, 128], f32, tag="ps_bcS")
            nc.tensor.matmul(ps_bc[:].rearrange("p k w -> p (k w)"), lhsT=onesT[:],
                             rhs=crow[:], start=True, stop=True)
            for k in range(ns):
                nc.vector.tensor_scalar(K0S[:, k, :], K0S[:, k, :], uS[:, k:k + 1],
                                        None, op0=OP.mult)
            nc.vector.tensor_tensor(K0S[:], K0S[:], ps_bc[:], op=OP.mult)
            ps_tt = pp.tile([2, ns * 128], f32, tag="ps_ttS")
            for k in range(ns):
                nc.tensor.matmul(ps_tt[:, k * 128:(k + 1) * 128],
                                 lhsT=tt_in[:, k, :], rhs=K0S[:, k, :],
                                 start=True, stop=True)
            ttsb = spool.tile([2, ns * 128], f32, tag="ttsbS")
            nc.vector.tensor_copy(ttsb[:], ps_tt[:])
            nc.sync.dma_start(tt_s_out_h[:], ttsb[:])
            nc.sync.dma_start(perm_s_h[:], K0S[:])

    out_names = [f"perm_big{g['b']}" for g in bigs] + \
                [f"ttout_big{g['b']}" for g in bigs]
    if ns:
        out_names += ["perm_small", "ttout_small"]
    return nc, out_names


# ---------------------------------------------------------------- host side
_compiled = {}


def kernel(event_time, event_type, clean_enc_out, seq_lens, gumbel_noise,
           W1, b1, W2, b2):
    _install_wait_fix()
    event_time = np.asarray(event_time, dtype=np.float32)
    event_type_f = np.asarray(event_type).astype(np.float32)
    x = np.asarray(clean_enc_out, dtype=np.float32)
    s_arr = [int(v) for v in np.asarray(seq_lens)]
    g_all = np.asarray(gumbel_noise, dtype=np.float32)
    W1 = np.asarray(W1, dtype=np.float32)
    b1 = np.asarray(b1, dtype=np.float32)
    W2 = np.asarray(W2, dtype=np.float32)
    b2 = np.asarray(b2, dtype=np.float32)

    plan = _plan(s_arr)
    key = tuple(s_arr)
    if key not in _compiled:
        _compiled[key] = _build(plan)
    nc, out_names = _compiled[key]

    ns, Qs = plan["ns"], plan["Qs"]
    eye = np.eye(128, dtype=np.float32)
    shared = {
        "w1a": np.ascontiguousarray(W1[:D]),
        "w1b": np.ascontiguousarray(W1[D:]),
        "b1v": np.ascontiguousarray(b1[:, None]),
        "w2tau": np.ascontiguousarray(W2[:, 0:1] / TAU),
        "eye": eye,
    }
    gb2tau = (b2[0] / TAU).astype(np.float32)

    def gum_block(b, rows, W, s):
        # (gumbel + b2)/tau for valid (row<s, col<s); NEG_FILL elsewhere
        out = np.full((len(rows), W), NEG_FILL, dtype=np.float32)
        for q, r in enumerate(rows):
            if r < s:
                out[q, :s] = g_all[b, r, :s] / TAU + gb2tau
        return out

    big_static = []
    for g in plan["bigs"]:
        b, s, Q, W, rt = g["b"], g["s"], g["Q"], g["We"], g["rt"]
        xT = np.zeros((D, W), dtype=np.float32)
        xT[:, :s] = x[b, :s].T
        tt = np.zeros((128, rt, 2), dtype=np.float32)
        for r in range(s):
            tt[r % 128, r // 128, 0] = event_type_f[b, r]
            tt[r % 128, r // 128, 1] = event_time[b, r]
        big_static.append({"xT": xT, "tt": tt})

    if ns:
        tts = np.zeros((128, ns, 2), dtype=np.float32)
        for k, sm in enumerate(plan["smalls"]):
            b, s = sm["b"], sm["s"]
            tts[:s, k, 0] = event_type_f[b, :s]
            tts[:s, k, 1] = event_time[b, :s]

    in_maps = []
    for c in range(NCORES):
        m = dict(shared)
        for g, st in zip(plan["bigs"], big_static):
            b, s, Q, W = g["b"], g["s"], g["Q"], g["We"]
            i = b
            rows = list(range(c * Q, (c + 1) * Q))
            xTc = np.zeros((D, Q), dtype=np.float32)
            for q, r in enumerate(rows):
                if r < s:
                    xTc[:, q] = x[b, r]
            m[f"xT_big{i}"] = st["xT"]
            m[f"xTc_big{i}"] = xTc
            m[f"g_big{i}"] = gum_block(b, rows, W, s)
            m[f"tt_big{i}"] = st["tt"]
        if ns:
            k, ci = plan["core_small"][c]
            sm = plan["smalls"][k]
            b, s = sm["b"], sm["s"]
            WeS = plan["WeS"]
            xTs = np.zeros((D, WeS), dtype=np.float32)
            xTs[:, :s] = x[b, :s].T
            rows = list(range(ci * Qs, (ci + 1) * Qs))
            xTcs = np.zeros((D, Qs), dtype=np.float32)
            for q, r in enumerate(rows):
                if r < s:
                    xTcs[:, q] = x[b, r]
            m["xT_small"] = xTs
            m["xTc_small"] = xTcs
            m["g_small"] = gum_block(b, rows, WeS, s)
            m["tt_small"] = tts
        in_maps.append(m)

    if os.environ.get("TRNK_BACKEND") == "sim":
        from concourse.bass_interp import MultiCoreSim
        sim = MultiCoreSim(nc, NCORES)
        for c, m in enumerate(in_maps):
            for k, v in m.items():
                sim.cores[c].tensor(k)[:] = v
        sim.simulate()
        r0 = {name: np.array(sim.cores[0].tensor(name)) for name in out_names}
    else:
        res = run_bass_kernel_spmd(nc, in_maps, core_ids=list(range(NCORES)),
                                   trace=bool(os.environ.get("TRNK_TRACE")))
        if res.exec_time_ns is not None:
            kernel.last_exec_time_ns = res.exec_time_ns
            kernel.last_profile = res
        r0 = res.results[0]

    perm = np.zeros((B, L, L), dtype=np.float32)
    tp = np.zeros((B, L), dtype=np.float32)
    tm = np.zeros((B, L), dtype=np.float32)
    for g in plan["bigs"]:
        b, s, rt, W = g["b"], g["s"], g["rt"], g["W"]
        blk = r0[f"perm_big{b}"].transpose(1, 0, 2).reshape(rt * 128, W)
        perm[b, :s, :s] = blk[:s, :s]
        tp[b, :s] = r0[f"ttout_big{b}"][0, :s]
        tm[b, :s] = r0[f"ttout_big{b}"][1, :s]
    if ns:
        for k, sm in enumerate(plan["smalls"]):
            b, s = sm["b"], sm["s"]
            perm[b, :s, :s] = r0["perm_small"][:s, k, :s]
            tp[b, :s] = r0["ttout_small"][0, k * 128:k * 128 + s]
            tm[b, :s] = r0["ttout_small"][1, k * 128:k * 128 + s]
    return tp, tm, perm


kernel.last_exec_time_ns = None
kernel.last_profile = None


# revision 39
# speedup vs baseline: 1.0348x; 1.0027x over previous
"""Trainium2 Bass kernel for nn_AdversarialGenerator (gumbel-sinkhorn permutation).

Contract: kernel(**inputs) takes FULL numpy inputs, returns
(types_permed [B,L] f32, times_permed [B,L] f32, perm [B,L,L] f32).

Strategy (8 NeuronCores, one SPMD launch):
  - Specializes the program on the runtime seq_lens values: only the valid
    s_b x s_b block of each batch is computed; everything else is exactly 0
    (host pads gumbel with -1e30 so exp() kills padding).
  - Phase A: scores + exp, data-parallel over rows across all 8 cores.
    relu rows [H=128, W] on DVE/ACT; the W2-dot runs on the PE via a
    sliding-window masked-weight matrix so score rows accumulate row-major
    in PSUM (f32r for 1 cycle/row).
  - AllGather of the exp(score+gumbel) chunks (ordered with add_dep_helper).
  - Phase B: Sinkhorn in scaling-vector form r=1/(Kc), c=1/(K^T r) -- all
    chunked PE matmuls with [128,k] vector layouts and tiny DVE reciprocals.
    Replicated on every core (cheaper than per-iteration collectives).
  - Phase C: perm = diag(r) K diag(c); type/time einsums on PE; compact
    DMA outputs; host pastes into zeros.
"""
import os
import sys

sys.path.insert(0, "/opt/trn_rl_repo")

from contextlib import ExitStack

import numpy as np
import orjson

import concourse.bass as bass
import concourse.tile as tile
from concourse import mybir
from concourse.bass_utils import run_bass_kernel_spmd
from concourse.tile_rust import add_dep_helper

f32 = mybir.dt.float32
f32r = mybir.dt.float32r
AF = mybir.ActivationFunctionType
OP = mybir.AluOpType

B, L, D, H = 4, 512, 64, 128
TAU = 0.5
ITERS = 10
NCORES = 8
ACT_NUM = 5
ACT_DEN = 11
RELU_BUFS = 10
PSW_BUFS = 4
PHASE_LIMIT = "all"


class _PhaseDone(Exception):
    pass


_build_partial_result = None


def _set_partial(nc):
    global _build_partial_result
    _build_partial_result = (nc, [])

NEG_FILL = -1e30
EPS = 1e-30


# ---------------------------------------------------------------- wait fix
# This container's walrus accepts at most ONE sync wait per instruction.
# Tile attaches several; split the excess onto EventSemaphore carriers
# inserted right before the offender (same engine => same order).
def _legalize_bir_waits(bir: dict, max_waits: int = 1) -> int:
    n = 0
    for func in bir.get("functions", []):
        for bb in func.get("blocks", []):
            out = []
            for ins in bb.get("instructions", []):
                si = ins.get("sync_info")
                waits = (si or {}).get("on_wait") or []
                if len(waits) > max_waits:
                    excess, keep = waits[:-max_waits], waits[-max_waits:]
                    for k, w in enumerate(excess):
                        out.append({
                            "name": f"{ins['name']}_xw{k}",
                            "opcode": "EventSemaphore",
                            "engine": ins["engine"],
                            "ins": [],
                            "outs": [],
                            "sync_info": {"on_wait": [w], "on_update": []},
                            "debug": ins.get("debug"),
                        })
                        n += 1
                    si["on_wait"] = keep
                out.append(ins)
            bb["instructions"] = out
    return n


_patched = False


def _install_wait_fix():
    global _patched
    if _patched:
        return
    _patched = True
    import concourse.bass_utils as bu
    import concourse.bass2jax as b2j

    orig = bu.compile_bir_kernel

    def patched(bir_json, tmpdir, neff_name="file.neff"):
        bir = orjson.loads(bir_json)
        if _legalize_bir_waits(bir):
            bir_json = orjson.dumps(bir)
        return orig(bir_json, tmpdir, neff_name=neff_name)

    bu.compile_bir_kernel = patched
    b2j.compile_bir_kernel = patched


# ---------------------------------------------------------------- layout plan
def _plan(s_list):
    cdiv = lambda a, b: (a + b - 1) // b
    bigs, smalls = [], []
    for b in range(len(s_list)):
        (bigs if s_list[b] > 128 else smalls).append(b)

    plan = {"bigs": [], "smalls": [], "s": list(s_list)}
    for b in bigs:
        s = s_list[b]
        Q = cdiv(s, NCORES)          # rows per core
        R = NCORES * Q               # gathered rows (>= s)
        rt = cdiv(R, 128)            # row tiles
        ct = cdiv(s, 128)            # col tiles
        plan["bigs"].append({"b": b, "s": s, "Q": Q, "R": R, "rt": rt,
                             "ct": ct, "W": ct * 128, "We": 2 * cdiv(s, 2)})
    ns = len(smalls)
    if ns:
        best = None
        import itertools
        for alloc in itertools.product(range(1, NCORES + 1), repeat=ns):
            if sum(alloc) > NCORES:
                continue
            if any(a * cdiv(s_list[b], a) > 128 for a, b in zip(alloc, smalls)):
                continue
            qs = max(cdiv(s_list[b], a) for a, b in zip(alloc, smalls))
            key = (qs, sum(alloc))
            if best is None or key < best[0]:
                best = (key, alloc)
        alloc = list(best[1])
        Qs = max(cdiv(s_list[b], a) for a, b in zip(alloc, smalls))
        # core -> (small index k, chunk index ci); unassigned cores mirror k=0,ci=0
        core_small = [(0, 0)] * NCORES
        cidx = 0
        for k, (a, b) in enumerate(zip(alloc, smalls)):
            for ci in range(a):
                core_small[cidx] = (k, ci)
                cidx += 1
        plan["smalls"] = [{"b": b, "s": s_list[b], "alloc": a}
                          for a, b in zip(alloc, smalls)]
        plan["Qs"] = Qs
        plan["core_small"] = core_small
        plan["ns"] = ns
        plan["WeS"] = 2 * cdiv(max(s_list[b] for b in smalls), 2)
    else:
        plan["WeS"] = 0
        plan["Qs"] = 0
        plan["ns"] = 0
        plan["core_small"] = [(0, 0)] * NCORES
    return plan


# ---------------------------------------------------------------- builder
def _build(plan):
    try:
        return _build_inner(plan)
    except _PhaseDone:
        return _build_partial_result


def _build_inner(plan):
    global _build_partial_result
    nc = bass.Bass(num_devices=NCORES)
    ns, Qs = plan["ns"], plan["Qs"]
    bigs = plan["bigs"]

    dp = nc.declare_dram_parameter
    W1a_h = dp("w1a", [D, H], f32, isOutput=False)
    W1b_h = dp("w1b", [D, H], f32, isOutput=False)
    b1_h = dp("b1v", [H, 1], f32, isOutput=False)
    w2_h = dp("w2tau", [H, 1], f32, isOutput=False)
    eye_h = dp("eye", [128, 128], f32, isOutput=False)

    big_in = []
    for g in bigs:
        i = g["b"]
        big_in.append({
            "xT": dp(f"xT_big{i}", [D, g["We"]], f32, isOutput=False),
            "xTc": dp(f"xTc_big{i}", [D, g["Q"]], f32, isOutput=False),
            "g": dp(f"g_big{i}", [g["Q"], g["We"]], f32, isOutput=False),
            "tt": dp(f"tt_big{i}", [128, g["rt"], 2], f32, isOutput=False),
        })
    if ns:
        WeS = plan["WeS"]
        xT_s_h = dp("xT_small", [D, WeS], f32, isOutput=False)
        xTc_s_h = dp("xTc_small", [D, Qs], f32, isOutput=False)
        g_s_h = dp("g_small", [Qs, WeS], f32, isOutput=False)
        tt_s_h = dp("tt_small", [128, ns, 2], f32, isOutput=False)

    big_out = []
    for g in bigs:
        i = g["b"]
        big_out.append({
            "perm": dp(f"perm_big{i}", [128, g["rt"], g["W"]], f32, isOutput=True),
            "tt": dp(f"ttout_big{i}", [2, g["W"]], f32, isOutput=True),
        })
    if ns:
        perm_s_h = dp("perm_small", [128, ns, 128], f32, isOutput=True)
        tt_s_out_h = dp("ttout_small", [2, ns * 128], f32, isOutput=True)

    big_sz = sum(g["Q"] * g["We"] for g in bigs)
    small_sz = Qs * plan["WeS"] if ns else 0
    cc_local_b = nc.dram_tensor("cc_local_b", [max(big_sz, 1)], f32)
    cc_gath_b = nc.dram_tensor("cc_gath_b", [NCORES, max(big_sz, 1)], f32,
                               addr_space="Shared")
    if ns:
        cc_local_s = nc.dram_tensor("cc_local_s", [small_sz], f32)
        cc_gath_s = nc.dram_tensor("cc_gath_s", [NCORES, small_sz], f32,
                                   addr_space="Shared")

    with ExitStack() as ctx:
        tc = ctx.enter_context(tile.TileContext(nc))
        const = ctx.enter_context(tc.tile_pool(name="const", bufs=1))
        setup = ctx.enter_context(tc.tile_pool(name="setup", bufs=2))
        rpool = ctx.enter_context(tc.tile_pool(name="relu", bufs=RELU_BUFS))
        spool = ctx.enter_context(tc.tile_pool(name="scratch", bufs=3))
        kpool = ctx.enter_context(tc.tile_pool(name="kmat", bufs=1))
        vpool = ctx.enter_context(tc.tile_pool(name="vecs", bufs=2))
        pp = ctx.enter_context(tc.tile_pool(name="ps", bufs=2, space="PSUM"))
        ppk = ctx.enter_context(tc.tile_pool(name="psk", bufs=2, space="PSUM"))

        # constants
        W1a = const.tile([D, H], f32)
        W1b = const.tile([D, H], f32)
        b1v = const.tile([H, 1], f32)
        eye = const.tile([128, 128], f32)
        w2sb = const.tile([H, 1], f32)
        zw2 = const.tile([H, 129], f32r)      # cols 0..127 zero, col 128 = W2/tau
        onesT = const.tile([1, 128], f32)
        nc.sync.dma_start(W1a[:], W1a_h[:])
        nc.sync.dma_start(W1b[:], W1b_h[:])
        nc.sync.dma_start(b1v[:], b1_h[:])
        nc.sync.dma_start(eye[:], eye_h[:])
        nc.sync.dma_start(w2sb[:], w2_h[:])
        nc.vector.memset(zw2[:, 0:128].bitcast(f32), 0.0)
        nc.vector.tensor_copy(zw2[:, 128:129], w2sb[:])   # f32 -> f32r round
        nc.vector.memset(onesT[:], 1.0)

        # ---------------- phase A: P0 chunks
        def phase_a(xT_h, xTc_h, g_h, Q, W, act_num, act_den, ccbuf, off):
            xT = setup.tile([D, W], f32, tag="xT")
            xTc = setup.tile([D, Q], f32, tag="xTc")
            gsb = setup.tile([Q, W], f32, tag="gsb")
            nc.sync.dma_start(xT[:], xT_h[:])
            nc.sync.dma_start(xTc[:], xTc_h[:])
            nc.sync.dma_start(gsb[:], g_h[:])

            ps_pj = pp.tile([H, W], f32, tag="pjpsum")
            nc.tensor.matmul(ps_pj[:], lhsT=W1b[:], rhs=xT[:], start=True, stop=True)
            pjT = setup.tile([H, W], f32, tag="pjT")
            nc.scalar.copy(pjT[:], ps_pj[:])

            ps_pi = pp.tile([H, Q], f32, tag="pipsum")
            nc.tensor.matmul(ps_pi[:], lhsT=W1a[:], rhs=xTc[:], start=True, stop=True)
            piTb = setup.tile([H, Q], f32, tag="piTb")
            nc.vector.tensor_scalar(piTb[:], ps_pi[:], b1v[:, 0:1], None, op0=OP.add)

            ps_sc = pp.tile([128, W], f32, tag="scpsum")
            for q in reversed(range(Q)):
                rt_ = rpool.tile([H, W], f32r, tag="relu_t")
                if act_num and (q % act_den) < act_num:
                    nc.scalar.activation(rt_[:], pjT[:], AF.Relu,
                                         bias=piTb[:, q:q + 1], scale=1.0)
                else:
                    nc.vector.tensor_scalar(rt_[:], pjT[:], piTb[:, q:q + 1], 0.0,
                                            op0=OP.add, op1=OP.max)
                nc.tensor.matmul(ps_sc[0:q + 1, :],
                                 lhsT=zw2[:, 128 - q:129],
                                 rhs=rt_[:],
                                 start=(q == Q - 1), stop=(q == 0),
                                 skip_group_check=True)
            t = spool.tile([Q, W], f32, tag="p0chunk")
            nc.vector.tensor_tensor(t[:], ps_sc[0:Q, :], gsb[:], op=OP.add)
            nc.scalar.activation(t[:], t[:], AF.Exp, bias=0.0, scale=1.0)
            st = nc.sync.dma_start(
                ccbuf[off:off + Q * W].rearrange("(q w) -> q w", w=W), t[:])
            return st

        # smalls first so their (smaller) collective hides under the big loop
        if ns:
            st_s = phase_a(xT_s_h, xTc_s_h, g_s_h, Qs, plan["WeS"], 0, 1,
                           cc_local_s, 0)
        big_stores = []
        off = 0
        for g, ih in zip(bigs, big_in):
            big_stores.append(phase_a(ih["xT"], ih["xTc"], ih["g"], g["Q"],
                                      g["We"], ACT_NUM, ACT_DEN, cc_local_b, off))
            off += g["Q"] * g["We"]

        # ---------------- allgathers
        if PHASE_LIMIT == "A":
            _set_partial(nc)
            raise _PhaseDone()
        grp = [list(range(NCORES))]
        if ns:
            cc_s = nc.gpsimd.collective_compute(
                "AllGather", OP.bypass, replica_groups=grp,
                ins=[cc_local_s[:]], outs=[cc_gath_s[:]])
            add_dep_helper(cc_s.ins, st_s.ins, sync=True,
                           reason="cc_s waits small chunk store")
        cc_b = None
        if bigs:
            cc_b = nc.gpsimd.collective_compute(
                "AllGather", OP.bypass, replica_groups=grp,
                ins=[cc_local_b[:]], outs=[cc_gath_b[:]])
            for st in big_stores:
                add_dep_helper(cc_b.ins, st.ins, sync=True,
                               reason="cc_b waits big chunk store")

        _ld_engines = [nc.sync, nc.scalar]
        _ld_big_engines = [nc.sync, nc.scalar, nc.gpsimd]
        _ld_rr = [0]

        def gathered_load(gath, cc, dst_ap, core, start_el, q0, nrows, W,
                          big=False):
            src = gath[core, start_el + q0 * W: start_el + (q0 + nrows) * W]
            engines = _ld_big_engines if big else _ld_engines
            eng = engines[_ld_rr[0] % len(engines)]
            _ld_rr[0] += 1
            ld = eng.dma_start(dst_ap, src.rearrange("(q w) -> q w", w=W))
            add_dep_helper(ld.ins, cc.ins, sync=True, reason="load waits cc")

        if PHASE_LIMIT == "gather":
            _set_partial(nc)
            raise _PhaseDone()
        # ---------------- assemble K0 matrices (smalls first)
        if ns:
            WeS = plan["WeS"]
            K0S = kpool.tile([128, ns, 128], f32, tag="K0S")
            nc.vector.memset(K0S[:], 0.0)
            for c in range(NCORES):
                k, ci = plan["core_small"][c]
                if c >= sum(sm["alloc"] for sm in plan["smalls"]):
                    continue
                p0 = ci * Qs
                gathered_load(cc_gath_s, cc_s, K0S[p0:p0 + Qs, k, 0:WeS],
                              c, 0, 0, Qs, WeS)
        big_k = []
        off = 0
        for g in bigs:
            Q, W, We, rt, ct = g["Q"], g["W"], g["We"], g["rt"], g["ct"]
            K0 = kpool.tile([128, rt, W], f32, tag=f"K0_{g['b']}")
            nc.vector.memset(K0[:], 0.0)
            for c in range(NCORES):
                q0 = 0
                while q0 < Q:
                    grow = c * Q + q0
                    tile_i, p0 = grow // 128, grow % 128
                    nrows = min(Q - q0, 128 - p0)
                    gathered_load(cc_gath_b, cc_b, K0[p0:p0 + nrows, tile_i, 0:We],
                                  c, off, q0, nrows, We, big=True)
                    q0 += nrows
            big_k.append(K0)
            off += Q * We

        if PHASE_LIMIT == "k0":
            _set_partial(nc)
            raise _PhaseDone()
        # ---------------- transposes -> KT0 (smalls first)
        if ns:
            KT0S = kpool.tile([128, ns, 128], f32, tag="KT0S")
            for k in range(ns):
                psT = ppk.tile([128, 128], f32, tag="w")
                nc.tensor.transpose(psT[:], K0S[:, k, :], eye[:])
                nc.vector.tensor_copy(KT0S[:, k, :], psT[:])
        big_kt = []
        for g, K0 in zip(bigs, big_k):
            rt, ct = g["rt"], g["ct"]
            KT0 = kpool.tile([128, ct, rt * 128], f32, tag=f"KT0_{g['b']}")
            for jc in range(ct):
                psT = ppk.tile([128, rt * 128], f32, tag="psT")
                for ic in range(rt):
                    nc.tensor.transpose(psT[:, ic * 128:(ic + 1) * 128],
                                        K0[:, ic, jc * 128:(jc + 1) * 128], eye[:])
                nc.vector.tensor_copy(KT0[:, jc, :], psT[:])
            big_kt.append(KT0)
        if ns:
            KT0S = kpool.tile([128, ns, 128], f32, tag="KT0S")
            for k in range(ns):
                psT = ppk.tile([128, 128], f32, tag="psTS")
                nc.tensor.transpose(psT[:], K0S[:, k, :], eye[:])
                nc.vector.tensor_copy(KT0S[:, k, :], psT[:])

        # ---------------- sinkhorn (scaling-vector form)
        # Reference quirk (f32 absorption at -1e9): after row-normalization the
        # fully-masked rows become exp(0)=1.0, polluting every column sum by
        # exactly (L - s). Divisor recurrence: d_t = K^T u_t + (L-s) * d_{t-1}.
        big_uc = []
        for g in bigs:
            c_t = vpool.tile([128, g["ct"]], f32, tag=f"c_{g['b']}")
            nc.vector.memset(c_t[:], 1.0)
            d_t = vpool.tile([128, g["ct"]], f32, tag=f"d_{g['b']}")
            nc.vector.memset(d_t[:], 1.0)
            big_uc.append({"c": c_t, "d": d_t, "u": None, "pol": float(L - g["s"])})
        if ns:
            cS = vpool.tile([128, ns], f32, tag="cS")
            nc.vector.memset(cS[:], 1.0)
            dS = vpool.tile([128, ns], f32, tag="dS")
            nc.vector.memset(dS[:], 1.0)
            polS = const.tile([128, ns], f32)
            for k, sm in enumerate(plan["smalls"]):
                nc.vector.memset(polS[:, k:k + 1], float(L - sm["s"]))
            uS = None

        for it in range(ITERS):
            for g, K0, KT0, uc in zip(bigs, big_k, big_kt, big_uc):
                rt, ct = g["rt"], g["ct"]
                ps_u = pp.tile([128, rt], f32, tag="ps_u")
                for ic in range(rt):
                    for jc in range(ct):
                        nc.tensor.matmul(ps_u[:, ic:ic + 1],
                                         lhsT=KT0[:, jc, ic * 128:(ic + 1) * 128],
                                         rhs=uc["c"][:, jc:jc + 1],
                                         start=(jc == 0), stop=(jc == ct - 1))
                tmp = spool.tile([128, rt], f32, tag="tmp_u")
                nc.vector.tensor_scalar(tmp[:], ps_u[:], EPS, None, op0=OP.add)
                u_t = vpool.tile([128, rt], f32, tag=f"u_{g['b']}")
                nc.vector.reciprocal(u_t[:], tmp[:])
                uc["u"] = u_t

                ps_c = pp.tile([128, ct], f32, tag="ps_c")
                for jc in range(ct):
                    for ic in range(rt):
                        nc.tensor.matmul(ps_c[:, jc:jc + 1],
                                         lhsT=K0[:, ic, jc * 128:(jc + 1) * 128],
                                         rhs=u_t[:, ic:ic + 1],
                                         start=(ic == 0), stop=(ic == rt - 1))
                dscale = spool.tile([128, ct], f32, tag="tmp_c")
                nc.vector.tensor_scalar(dscale[:], uc["d"][:], uc["pol"], None,
                                        op0=OP.mult)
                d_t = vpool.tile([128, ct], f32, tag=f"d_{g['b']}")
                nc.vector.tensor_tensor(d_t[:], ps_c[:], dscale[:], op=OP.add)
                uc["d"] = d_t
                c_t = vpool.tile([128, ct], f32, tag=f"c_{g['b']}")
                nc.vector.reciprocal(c_t[:], d_t[:])
                uc["c"] = c_t

            if ns:
                ps_uS = pp.tile([128, ns], f32, tag="ps_uS")
                for k in range(ns):
                    nc.tensor.matmul(ps_uS[:, k:k + 1], lhsT=KT0S[:, k, :],
                                     rhs=cS[:, k:k + 1], start=True, stop=True)
                tmpS = spool.tile([128, ns], f32, tag="tmp_uS")
                nc.vector.tensor_scalar(tmpS[:], ps_uS[:], EPS, None, op0=OP.add)
                uS = vpool.tile([128, ns], f32, tag="uS")
                nc.vector.reciprocal(uS[:], tmpS[:])

                ps_cS = pp.tile([128, ns], f32, tag="ps_cS")
                for k in range(ns):
                    nc.tensor.matmul(ps_cS[:, k:k + 1], lhsT=K0S[:, k, :],
                                     rhs=uS[:, k:k + 1], start=True, stop=True)
                dscaleS = spool.tile([128, ns], f32, tag="tmp_cS")
                nc.vector.tensor_tensor(dscaleS[:], dS[:], polS[:], op=OP.mult)
                dS = vpool.tile([128, ns], f32, tag="dS")
                nc.vector.tensor_tensor(dS[:], ps_cS[:], dscaleS[:], op=OP.add)
                cS = vpool.tile([128, ns], f32, tag="cS")
                nc.vector.reciprocal(cS[:], dS[:])

        if PHASE_LIMIT == "sink":
            _set_partial(nc)
            raise _PhaseDone()
        # ---------------- final perm + einsums
        def finalize(K0, u_t, c_t, rt, ct, W, tt_in, perm_out, tt_out):
            ps_crow = pp.tile([1, W], f32, tag="ps_crow")
            for jc in range(ct):
                nc.tensor.transpose(ps_crow[0:1, jc * 128:(jc + 1) * 128],
                                    c_t[:, jc:jc + 1], eye[:])
            crow = spool.tile([1, W], f32, tag="crow")
            nc.vector.tensor_copy(crow[:], ps_crow[:])
            ps_bc = pp.tile([128, W], f32, tag="ps_bc")
            nc.tensor.matmul(ps_bc[:], lhsT=onesT[:], rhs=crow[:], start=True,
                             stop=True)
            for ic in range(rt):
                nc.vector.tensor_scalar(K0[:, ic, :], K0[:, ic, :],
                                        u_t[:, ic:ic + 1], None, op0=OP.mult)
                nc.vector.tensor_tensor(K0[:, ic, :], K0[:, ic, :], ps_bc[:],
                                        op=OP.mult)
            ps_tt = pp.tile([2, W], f32, tag="ps_tt")
            for ic in range(rt):
                nc.tensor.matmul(ps_tt[:], lhsT=tt_in[:, ic, :], rhs=K0[:, ic, :],
                                 start=(ic == 0), stop=(ic == rt - 1))
            ttsb = spool.tile([2, W], f32, tag="ttsb")
            nc.vector.tensor_copy(ttsb[:], ps_tt[:])
            nc.sync.dma_start(tt_out[:], ttsb[:])
            nc.sync.dma_start(perm_out[:], K0[:])

        for g, K0, uc, ih, oh in zip(bigs, big_k, big_uc, big_in, big_out):
            tt_in = setup.tile([128, g["rt"], 2], f32, tag="ttbig")
            nc.sync.dma_start(tt_in[:], ih["tt"][:])
            finalize(K0, uc["u"], uc["c"], g["rt"], g["ct"], g["W"],
                     tt_in, oh["perm"], oh["tt"])
        if ns:
            tt_in = setup.tile([128, ns, 2], f32, tag="ttsmall")
            nc.sync.dma_start(tt_in[:], tt_s_h[:])
            ps_crow = pp.tile([1, ns * 128], f32, tag="ps_crowS")
            for k in range(ns):
                nc.tensor.transpose(ps_crow[0:1, k * 128:(k + 1) * 128],
                                    cS[:, k:k + 1], eye[:])
            crow = spool.tile([1, ns * 128], f32, tag="crowS")
            nc.vector.tensor_copy(crow[:], ps_crow[:])
            ps_bc = pp.tile([128, ns, 128], f32, tag="ps_bcS")
            nc.tensor.matmul(ps_bc[:].rearrange("p k w -> p (k w)"), lhsT=onesT[:],
                             rhs=crow[:], start=True, stop=True)
            for k in range(ns):
                nc.vector.tensor_scalar(K0S[:, k, :], K0S[:, k, :], uS[:, k:k + 1],
                                        None, op0=OP.mult)
            nc.vector.tensor_tensor(K0S[:], K0S[:], ps_bc[:], op=OP.mult)
            ps_tt = pp.tile([2, ns * 128], f32, tag="ps_ttS")
            for k in range(ns):
                nc.tensor.matmul(ps_tt[:, k * 128:(k + 1) * 128],
                                 lhsT=tt_in[:, k, :], rhs=K0S[:, k, :],
                                 start=True, stop=True)
            ttsb = spool.tile([2, ns * 128], f32, tag="ttsbS")
            nc.vector.tensor_copy(ttsb[:], ps_tt[:])
            nc.sync.dma_start(tt_s_out_h[:], ttsb[:])
            nc.sync.dma_start(perm_s_h[:], K0S[:])

    out_names = [f"perm_big{g['b']}" for g in bigs] + \
                [f"ttout_big{g['b']}" for g in bigs]
    if ns:
        out_names += ["perm_small", "ttout_small"]
    return nc, out_names


# ---------------------------------------------------------------- host side
_compiled = {}


def kernel(event_time, event_type, clean_enc_out, seq_lens, gumbel_noise,
           W1, b1, W2, b2):
    _install_wait_fix()
    event_time = np.asarray(event_time, dtype=np.float32)
    event_type_f = np.asarray(event_type).astype(np.float32)
    x = np.asarray(clean_enc_out, dtype=np.float32)
    s_arr = [int(v) for v in np.asarray(seq_lens)]
    g_all = np.asarray(gumbel_noise, dtype=np.float32)
    W1 = np.asarray(W1, dtype=np.float32)
    b1 = np.asarray(b1, dtype=np.float32)
    W2 = np.asarray(W2, dtype=np.float32)
    b2 = np.asarray(b2, dtype=np.float32)

    plan = _plan(s_arr)
    key = tuple(s_arr)
    if key not in _compiled:
        _compiled[key] = _build(plan)
    nc, out_names = _compiled[key]

    ns, Qs = plan["ns"], plan["Qs"]
    eye = np.eye(128, dtype=np.float32)
    shared = {
        "w1a": np.ascontiguousarray(W1[:D]),
        "w1b": np.ascontiguousarray(W1[D:]),
        "b1v": np.ascontiguousarray(b1[:, None]),
        "w2tau": np.ascontiguousarray(W2[:, 0:1] / TAU),
        "eye": eye,
    }
    gb2tau = (b2[0] / TAU).astype(np.float32)

    def gum_block(b, rows, W, s):
        # (gumbel + b2)/tau for valid (row<s, col<s); NEG_FILL elsewhere
        out = np.full((len(rows), W), NEG_FILL, dtype=np.float32)
        for q, r in enumerate(rows):
            if r < s:
                out[q, :s] = g_all[b, r, :s] / TAU + gb2tau
        return out

    big_static = []
    for g in plan["bigs"]:
        b, s, Q, W, rt = g["b"], g["s"], g["Q"], g["We"], g["rt"]
        xT = np.zeros((D, W), dtype=np.float32)
        xT[:, :s] = x[b, :s].T
        tt = np.zeros((128, rt, 2), dtype=np.float32)
        for r in range(s):
            tt[r % 128, r // 128, 0] = event_type_f[b, r]
            tt[r % 128, r // 128, 1] = event_time[b, r]
        big_static.append({"xT": xT, "tt": tt})

    if ns:
        tts = np.zeros((128, ns, 2), dtype=np.float32)
        for k, sm in enumerate(plan["smalls"]):
            b, s = sm["b"], sm["s"]
            tts[:s, k, 0] = event_type_f[b, :s]
            tts[:s, k, 1] = event_time[b, :s]

    in_maps = []
    for c in range(NCORES):
        m = dict(shared)
        for g, st in zip(plan["bigs"], big_static):
            b, s, Q, W = g["b"], g["s"], g["Q"], g["We"]
            i = b
            rows = list(range(c * Q, (c + 1) * Q))
            xTc = np.zeros((D, Q), dtype=np.float32)
            for q, r in enumerate(rows):
                if r < s:
                    xTc[:, q] = x[b, r]
            m[f"xT_big{i}"] = st["xT"]
            m[f"xTc_big{i}"] = xTc
            m[f"g_big{i}"] = gum_block(b, rows, W, s)
            m[f"tt_big{i}"] = st["tt"]
        if ns:
            k, ci = plan["core_small"][c]
            sm = plan["smalls"][k]
            b, s = sm["b"], sm["s"]
            WeS = plan["WeS"]
            xTs = np.zeros((D, WeS), dtype=np.float32)
            xTs[:, :s] = x[b, :s].T
            rows = list(range(ci * Qs, (ci + 1) * Qs))
            xTcs = np.zeros((D, Qs), dtype=np.float32)
            for q, r in enumerate(rows):
                if r < s:
                    xTcs[:, q] = x[b, r]
            m["xT_small"] = xTs
            m["xTc_small"] = xTcs
            m["g_small"] = gum_block(b, rows, WeS, s)
            m["tt_small"] = tts
        in_maps.append(m)

    if os.environ.get("TRNK_BACKEND") == "sim":
        from concourse.bass_interp import MultiCoreSim
        sim = MultiCoreSim(nc, NCORES)
        for c, m in enumerate(in_maps):
            for k, v in m.items():
                sim.cores[c].tensor(k)[:] = v
        sim.simulate()
        r0 = {name: np.array(sim.cores[0].tensor(name)) for name in out_names}
    else:
        res = run_bass_kernel_spmd(nc, in_maps, core_ids=list(range(NCORES)),
                                   trace=bool(os.environ.get("TRNK_TRACE")))
        if res.exec_time_ns is not None:
            kernel.last_exec_time_ns = res.exec_time_ns
            kernel.last_profile = res
        r0 = res.results[0]

    perm = np.zeros((B, L, L), dtype=np.float32)
    tp = np.zeros((B, L), dtype=np.float32)
    tm = np.zeros((B, L), dtype=np.float32)
    for g in plan["bigs"]:
        b, s, rt, W = g["b"], g["s"], g["rt"], g["W"]
        blk = r0[f"perm_big{b}"].transpose(1, 0, 2).reshape(rt * 128, W)
        perm[b, :s, :s] = blk[:s, :s]
        tp[b, :s] = r0[f"ttout_big{b}"][0, :s]
        tm[b, :s] = r0[f"ttout_big{b}"][1, :s]
    if ns:
        for k, sm in enumerate(plan["smalls"]):
            b, s = sm["b"], sm["s"]
            perm[b, :s, :s] = r0["perm_small"][:s, k, :s]
            tp[b, :s] = r0["ttout_small"][0, k * 128:k * 128 + s]
            tm[b, :s] = r0["ttout_small"][1, k * 128:k * 128 + s]
    return tp, tm, perm


kernel.last_exec_time_ns = None
kernel.last_profile = None
